# revision 1
# baseline (speedup 1.0000x reference)
"""Trainium2 Bass kernel for nn_Block_56126632624726 (dense transformer block).

Reference computation (fp32, B=4, L=2048, D=1024, H=8 heads, hd=128):
    h = LayerNorm(x) * gamma + beta
    [q,k,v,lin,pre] = h @ w_qkv.T            (5*D outputs)
    attn = causal p-softmax attention (p=2)
    branch = [lin * gelu(pre), attn]
    out = x + branch @ w_out.T

Sharding: 8 cores = 4 batches (data parallel) x 2 tensor-parallel halves.
Within a batch pair, core j in {0,1} owns heads 4j..4j+3 (512 cols of each
of q/k/v) plus lin/pre cols 512j..512j+512, and the matching w_out input
columns. Each core emits a partial [2048, 1024] output; the host sums the
two partials per batch and adds the residual x (so no device collectives).

Per-core kernel (all matmuls in fp32r = 1-pass FP22):
  - LayerNorm in dim-major layout: token sums via PE ones-matmuls, per-token
    scale/shift broadcast across partitions via a DRAM-bounce DMA.
  - Projections QT/KT (dim-major), V (token-major, bounced via DRAM), and
    gT = lin * gelu(pre) (dim-major).
  - Attention per head with transposed scores ST[k,q] (no extra transposes):
    f = exp(s) (no max subtraction: |2s| < 18 on normalized data, fp32-safe),
    causal masking via a precomputed mask template, r[q] = sum_k f^2 via PE
    ones-matmul, OT = V.T @ f accumulated in PSUM, attnT = OT * rsqrt(r).
  - Out-projection from branchT = [gT; attnT] with w_out slice.
"""

import numpy as np

# ---------------------------------------------------------------------------
# constants (hardcoded problem shapes)
# ---------------------------------------------------------------------------
B = 4
L = 2048
D = 1024
H = 8  # global heads
HD = 128
HL = 4  # heads per core
P = 128
KC = D // P  # 8 dim-chunks
NQ = 4  # token quarters
TQ = L // NQ  # 512
NT = L // P  # 16 token tiles
SCALE = float(HD) ** -0.5
EPS = 1e-5

_CACHED = {}


def _install_tile_drain_patch(tile, mybir):
    """walrus limits sem waits per SP CTRL instruction to 1; split the
    TileContext final drain's waits across sequential drain instructions."""
    from concourse.vector_clock import ScopedClock

    if getattr(tile.TileContext, "_drain_patched", False):
        return

    def _patched(self, tick_clock, wait_clock):
        drain_inst = self.nc.sync.drain()
        wait_clock.add_sem_waits(
            drain_inst.ins, ScopedClock({None: tick_clock.global_clock})
        )
        si = drain_inst.ins.sync_info
        waits = list(si.on_wait or []) if si else []
        if len(waits) > 1:
            si.on_wait = waits[:1]
            for w in waits[1:]:
                d2 = self.nc.sync.drain()
                d2.ins.sync_info = mybir.SyncInfo(on_wait=[w], on_update=[])
        self.nc.all_engine_barrier()
        popped = self.nc._tile_sem_poison_stack.pop()
        assert popped is self._sem_poison
        self.nc.clear_and_free_semaphores(list(self.sems.allocated().values()))
        self.nc.all_engine_barrier()

    tile.TileContext._drain_and_barrier = _patched
    tile.TileContext._drain_patched = True


def _split_multi_waits(nc, mybir):
    """This walrus build supports at most ONE sync-wait per instruction
    (single wait slot in every engine's 64B encoding). Tile's wait assignment
    can attach several. Engine streams execute in order (including SP's DMA
    triggers), so move extra waits onto same-engine nops inserted before the
    instruction."""
    eng_builder = {
        mybir.EngineType.PE: nc.tensor,
        mybir.EngineType.DVE: nc.vector,
        mybir.EngineType.Activation: nc.scalar,
        mybir.EngineType.SP: nc.sync,
        mybir.EngineType.Pool: nc.gpsimd,
    }

    def make_nop(engine):
        bi = eng_builder[engine].nop(nofuse=True)
        inst = bi.ins
        nc.cur_bb.bb.instructions.remove(inst)
        return inst

    for f in nc.m.functions:
        for bb in f.blocks:
            insts = bb.instructions
            rebuilt = []
            changed = False
            for inst in list(insts):
                si = inst.sync_info
                waits = list(si.on_wait or []) if si else []
                if len(waits) > 1:
                    changed = True
                    for w in waits[:-1]:
                        nop = make_nop(inst.engine)
                        nop.sync_info = mybir.SyncInfo(on_wait=[w], on_update=[])
                        rebuilt.append(nop)
                    si.on_wait = waits[-1:]
                rebuilt.append(inst)
            if changed:
                insts.clear()
                insts.extend(rebuilt)


def _build_nc():
    import concourse.bass as bass
    import concourse.tile as tile
    from concourse import mybir

    _install_tile_drain_patch(tile, mybir)

    f32 = mybir.dt.float32
    f32r = mybir.dt.float32r
    AF = mybir.ActivationFunctionType
    OP = mybir.AluOpType

    def rr(ap):
        return ap.bitcast(f32r)

    nc = bass.Bass()

    xT = nc.declare_dram_parameter("xT", [D, L], f32, isOutput=False)
    wq = nc.declare_dram_parameter("wqT", [D, 512], f32, isOutput=False)
    wk = nc.declare_dram_parameter("wkT", [D, 512], f32, isOutput=False)
    wv = nc.declare_dram_parameter("wvT", [D, 512], f32, isOutput=False)
    wl = nc.declare_dram_parameter("wlinT", [D, 512], f32, isOutput=False)
    wp = nc.declare_dram_parameter("wpreT", [D, 512], f32, isOutput=False)
    wo = nc.declare_dram_parameter("woT", [D, D], f32, isOutput=False)
    gamma = nc.declare_dram_parameter("gamma128", [P, KC], f32, isOutput=False)
    beta = nc.declare_dram_parameter("beta128", [P, KC], f32, isOutput=False)
    maskT = nc.declare_dram_parameter("maskT", [P, 896], f32, isOutput=False)
    out = nc.declare_dram_parameter("out", [L, D], f32, isOutput=True)

    xT_r = xT.rearrange("(o p) t -> p o t", p=P)  # [128, 8, 2048]
    wq_r = wq.rearrange("(o p) f -> p o f", p=P)  # [128, 8, 512]
    wk_r = wk.rearrange("(o p) f -> p o f", p=P)
    wv_r = wv.rearrange("(o p) f -> p o f", p=P)
    wl_r = wl.rearrange("(o p) f -> p o f", p=P)
    wp_r = wp.rearrange("(o p) f -> p o f", p=P)
    wo_r = wo.rearrange("(o p) f -> p o f", p=P)  # [128, 8, 1024]

    with tile.TileContext(nc) as tc:
        with tc.tile_pool(name="persist", bufs=1) as persist:
            masks = persist.tile([P, 896], f32)
            nc.sync.dma_start(out=rr(masks[:]), in_=rr(maskT[:]))
            ones = masks[:, 600:601]  # all-ones column of the mask template
            epst = persist.tile([P, 1], f32)
            nc.vector.memset(epst, EPS)
            gam = persist.tile([P, KC], f32)
            nc.sync.dma_start(out=gam, in_=gamma[:])
            bet = persist.tile([P, KC], f32)
            nc.sync.dma_start(out=bet, in_=beta[:])
            gT = persist.tile([P, HL, L], f32)  # lin*gelu(pre), dim-major

            qk = tc.alloc_tile_pool(name="qk", bufs=1)
            QT = qk.tile([P, HL, L], f32)
            KT = qk.tile([P, HL, L], f32)
            V = qk.tile([P, NT, 512], f32)  # token-major V

            # ---------------- Phase 1: LN + projections ----------------
            with (
                tc.tile_pool(name="hq", bufs=2) as hq,
                tc.tile_pool(name="wst", bufs=2) as wst,
                tc.tile_pool(name="wvs", bufs=3) as wvs,
                tc.tile_pool(name="ptmp", bufs=2) as ptmp,
                tc.tile_pool(name="bc1", bufs=1) as bc1,
                tc.tile_pool(name="rows", bufs=1) as rows,
                tc.tile_pool(name="rowd", bufs=4, space="DRAM") as rowd,
                tc.tile_pool(name="pps", bufs=4, space="PSUM") as pps,
                tc.tile_pool(name="rps", bufs=2, space="PSUM") as rps,
            ):
                for q in range(NQ):
                    tsl = slice(TQ * q, TQ * q + TQ)
                    h_sb = hq.tile([P, KC, TQ], f32, tag="h")
                    nc.sync.dma_start(out=rr(h_sb[:]), in_=rr(xT_r[:, :, tsl]))

                    # token sums and sum-of-squares via PE ones-matmuls
                    s1 = rps.tile([1, TQ], f32, tag="s1")
                    s2 = rps.tile([1, TQ], f32, tag="s2")
                    for k in range(KC):
                        nc.tensor.matmul(
                            s1, lhsT=rr(ones), rhs=rr(h_sb[:, k]),
                            start=(k == 0), stop=(k == KC - 1),
                        )
                    for k in range(KC):
                        x2 = ptmp.tile([P, TQ], f32, tag="x2")
                        nc.vector.tensor_mul(out=rr(x2[:]), in0=h_sb[:, k], in1=h_sb[:, k])
                        nc.tensor.matmul(
                            s2, lhsT=rr(ones), rhs=rr(x2[:]),
                            start=(k == 0), stop=(k == KC - 1),
                        )
                    # rows: mu, var, inv = rsqrt(var+eps), ninv = -mu*inv
                    mu = rows.tile([1, TQ], f32, tag="mu")
                    nc.scalar.mul(out=mu[:], in_=s1, mul=1.0 / D)
                    m2 = rows.tile([1, TQ], f32, tag="m2")
                    nc.scalar.mul(out=m2[:], in_=s2, mul=1.0 / D)
                    var = rows.tile([1, TQ], f32, tag="var")
                    nc.vector.tensor_mul(out=var[:], in0=mu[:], in1=mu[:])
                    nc.vector.tensor_tensor(
                        out=var[:], in0=m2[:], in1=var[:], op=OP.subtract
                    )
                    sd = rows.tile([1, TQ], f32, tag="sd")
                    nc.scalar.activation(
                        out=sd[:], in_=var[:], func=AF.Sqrt, bias=epst[:1, :]
                    )
                    inv = rows.tile([1, TQ], f32, tag="inv")
                    nc.vector.reciprocal(out=inv[:], in_=sd[:])
                    ninv = rows.tile([1, TQ], f32, tag="ninv")
                    nc.vector.tensor_mul(out=ninv[:], in0=mu[:], in1=inv[:])
                    nc.scalar.mul(out=ninv[:], in_=ninv[:], mul=-1.0)
                    # broadcast rows across partitions via DRAM bounce
                    import concourse.bass as _b

                    invb = bc1.tile([P, TQ], f32, tag="invb")
                    ninb = bc1.tile([P, TQ], f32, tag="ninb")
                    for row_t, bt in ((inv, invb), (ninv, ninb)):
                        rd = rowd.tile([1, TQ], f32, tag="rowd")
                        nc.sync.dma_start(out=rd, in_=row_t[:])
                        bap = _b.AP(tensor=rd.tensor, offset=rd.offset, ap=[[0, P], [1, TQ]])
                        nc.sync.dma_start(out=bt[:], in_=bap)
                    # normalize: h = (x*inv - mu*inv) * gamma + beta
                    for k in range(KC):
                        nc.vector.tensor_mul(out=rr(h_sb[:, k]), in0=h_sb[:, k], in1=invb[:])
                        nc.vector.tensor_add(out=rr(h_sb[:, k]), in0=h_sb[:, k], in1=ninb[:])
                        nc.vector.tensor_scalar(
                            out=rr(h_sb[:, k]), in0=h_sb[:, k],
                            scalar1=gam[:, k : k + 1], scalar2=bet[:, k : k + 1],
                            op0=OP.mult, op1=OP.add,
                        )

                    # QT / KT projections (dim-major)
                    for dst, wdram in ((QT, wq_r), (KT, wk_r)):
                        for m in range(HL):
                            wt = wst.tile([P, KC, P], f32, tag="w")
                            nc.sync.dma_start(out=rr(wt[:]), in_=rr(wdram[:, :, P * m : P * m + P]))
                            ps = pps.tile([P, TQ], f32, tag="mm")
                            for k in range(KC):
                                nc.tensor.matmul(
                                    ps, lhsT=rr(wt[:, k]), rhs=rr(h_sb[:, k]),
                                    start=(k == 0), stop=(k == KC - 1),
                                )
                            nc.vector.tensor_copy(out=rr(dst[:, m, tsl]), in_=ps)

                    # gT = lin * gelu(pre) (dim-major)
                    for c in range(HL):
                        wtp = wst.tile([P, KC, P], f32, tag="w")
                        nc.sync.dma_start(out=rr(wtp[:]), in_=rr(wp_r[:, :, P * c : P * c + P]))
                        psp = pps.tile([P, TQ], f32, tag="mm")
                        for k in range(KC):
                            nc.tensor.matmul(
                                psp, lhsT=rr(wtp[:, k]), rhs=rr(h_sb[:, k]),
                                start=(k == 0), stop=(k == KC - 1),
                            )
                        gel = ptmp.tile([P, TQ], f32, tag="gel")
                        nc.scalar.activation(out=gel[:], in_=psp, func=AF.Gelu)
                        wtl = wst.tile([P, KC, P], f32, tag="w")
                        nc.sync.dma_start(out=rr(wtl[:]), in_=rr(wl_r[:, :, P * c : P * c + P]))
                        psl = pps.tile([P, TQ], f32, tag="mm")
                        for k in range(KC):
                            nc.tensor.matmul(
                                psl, lhsT=rr(wtl[:, k]), rhs=rr(h_sb[:, k]),
                                start=(k == 0), stop=(k == KC - 1),
                            )
                        nc.vector.tensor_mul(out=rr(gT[:, c, tsl]), in0=psl, in1=gel[:])

                    # V projection (token-major), straight into SBUF.
                    # k-outer with streamed wv chunks; 4 concurrent psum accums.
                    vps = [
                        pps.tile([P, TQ], f32, tag="mm", name=f"vps{q}_{i}")
                        for i in range(NQ)
                    ]
                    for k in range(KC):
                        wvk = wvs.tile([P, TQ], f32, tag="wvk")
                        nc.sync.dma_start(out=rr(wvk[:]), in_=rr(wv_r[:, k, :]))
                        for i in range(NQ):
                            nc.tensor.matmul(
                                vps[i],
                                lhsT=rr(h_sb[:, k, P * i : P * i + P]),
                                rhs=rr(wvk[:]),
                                start=(k == 0), stop=(k == KC - 1),
                            )
                    for i in range(NQ):
                        nc.vector.tensor_copy(
                            out=rr(V[:, NQ * q + i, :]), in_=vps[i]
                        )

            # ---------------- Phase 2: attention ----------------
            attn_out = tc.alloc_tile_pool(name="attn_out", bufs=1)
            attnT = attn_out.tile([P, HL, L], f32)
            wop0 = tc.alloc_tile_pool(name="wop0", bufs=1)
            woh0 = wop0.tile([P, KC, 512], f32)
            nc.sync.dma_start(out=rr(woh0[:]), in_=rr(wo_r[:, :, 0:512]))
            import concourse.bass as _b

            with (
                tc.tile_pool(name="fp", bufs=3) as fp,
                tc.tile_pool(name="ep", bufs=2) as ep,
                tc.tile_pool(name="arow", bufs=2) as arow,
                tc.tile_pool(name="ard", bufs=2, space="DRAM") as ard,
                tc.tile_pool(name="cb", bufs=2) as cbp,
                tc.tile_pool(name="stps", bufs=3, space="PSUM") as stps,
                tc.tile_pool(name="ops", bufs=2, space="PSUM") as ops,
                tc.tile_pool(name="arps", bufs=2, space="PSUM") as arps,
            ):
                for h in range(HL):
                    for J in range(NQ):
                        jsl = slice(TQ * J, TQ * J + TQ)
                        nt = 4 * J + 4
                        o_ps = ops.tile([P, TQ], f32, tag="o")
                        r_ps = arps.tile([1, TQ], f32, tag="r")
                        for t in range(nt):
                            st_ps = stps.tile([P, TQ], f32, tag="st")
                            nc.tensor.matmul(
                                st_ps,
                                lhsT=rr(KT[:, h, P * t : P * t + P]),
                                rhs=rr(QT[:, h, jsl]),
                                start=True, stop=True,
                            )
                            f = fp.tile([P, TQ], f32, tag="f")
                            nc.scalar.activation(
                                out=rr(f[:]), in_=st_ps, func=AF.Exp, scale=SCALE
                            )
                            if t >= 4 * J:
                                off = 384 - P * (t - 4 * J)
                                nc.vector.tensor_mul(
                                    out=rr(f[:]), in0=f[:], in1=masks[:, off : off + TQ]
                                )
                            e = ep.tile([P, TQ], f32, tag="e")
                            nc.vector.tensor_mul(out=rr(e[:]), in0=f[:], in1=f[:])
                            nc.tensor.matmul(
                                r_ps, lhsT=rr(ones), rhs=rr(e[:]),
                                start=(t == 0), stop=(t == nt - 1),
                            )
                            nc.tensor.matmul(
                                o_ps,
                                lhsT=rr(V[:, t, P * h : P * h + P]),
                                rhs=rr(f[:]),
                                start=(t == 0), stop=(t == nt - 1),
                            )
                        # attnT[:, h, J] = o_ps * rsqrt(r)
                        sq = arow.tile([1, TQ], f32, tag="sq")
                        nc.scalar.activation(out=sq[:], in_=r_ps, func=AF.Sqrt)
                        cr = arow.tile([1, TQ], f32, tag="cr")
                        nc.vector.reciprocal(out=cr[:], in_=sq[:])
                        rd = ard.tile([1, TQ], f32, tag="ard")
                        nc.sync.dma_start(out=rd, in_=cr[:])
                        bap = _b.AP(tensor=rd.tensor, offset=rd.offset, ap=[[0, P], [1, TQ]])
                        cbt = cbp.tile([P, TQ], f32, tag="cb")
                        nc.sync.dma_start(out=cbt[:], in_=bap)
                        nc.vector.tensor_mul(
                            out=rr(attnT[:, h, jsl]), in0=o_ps, in1=cbt[:]
                        )

            # ---------------- Phase 3: out projection (n-outer) ----------------
            with (
                tc.tile_pool(name="wop1", bufs=1) as wop1,
                tc.tile_pool(name="obuf", bufs=3) as obuf,
                tc.tile_pool(name="ops3", bufs=4, space="PSUM") as ops3,
            ):
                woh1 = wop1.tile([P, KC, 512], f32)
                nc.sync.dma_start(out=rr(woh1[:]), in_=rr(wo_r[:, :, 512:1024]))
                for n, woh in ((0, woh0), (1, woh1)):
                    for i in range(NT):
                        ps = ops3.tile([P, 512], f32, tag="ops")
                        for c in range(KC):
                            src = gT if c < HL else attnT
                            nc.tensor.matmul(
                                ps,
                                lhsT=rr(src[:, c % HL, P * i : P * i + P]),
                                rhs=rr(woh[:, c]),
                                start=(c == 0), stop=(c == KC - 1),
                            )
                        ot = obuf.tile([P, 512], f32, tag="ot")
                        nc.vector.tensor_copy(out=ot[:], in_=ps)
                        nc.sync.dma_start(
                            out=out[P * i : P * i + P, 512 * n : 512 * n + 512],
                            in_=ot[:],
                        )
            wop0.release()
            attn_out.release()
            qk.release()

    _split_multi_waits(nc, mybir)
    return nc


def _core_inputs(inputs, core):
    """Build the per-core input map (numpy, host-side sharding/layout)."""
    x = np.ascontiguousarray(inputs["x"], dtype=np.float32)
    gamma = np.asarray(inputs["gamma"], dtype=np.float32)
    beta = np.asarray(inputs["beta"], dtype=np.float32)
    w_qkv = np.asarray(inputs["w_qkv"], dtype=np.float32)
    w_out = np.asarray(inputs["w_out"], dtype=np.float32)

    b, j = core // 2, core % 2
    sl = slice(512 * j, 512 * j + 512)
    xT = np.ascontiguousarray(x[b].T)
    wqT = np.ascontiguousarray(w_qkv[0 * D : 1 * D][sl].T)
    wkT = np.ascontiguousarray(w_qkv[1 * D : 2 * D][sl].T)
    wvT = np.ascontiguousarray(w_qkv[2 * D : 3 * D][sl].T)
    wlinT = np.ascontiguousarray(w_qkv[3 * D : 4 * D][sl].T)
    wpreT = np.ascontiguousarray(w_qkv[4 * D : 5 * D][sl].T)
    cols = np.r_[512 * j : 512 * j + 512, D + 512 * j : D + 512 * j + 512]
    woT = np.ascontiguousarray(w_out[:, cols].T)
    gamma128 = np.ascontiguousarray(gamma.reshape(KC, P).T)
    beta128 = np.ascontiguousarray(beta.reshape(KC, P).T)
    # transposed causal mask template: maskT[kk, c] = 1 iff c >= kk + 384
    kk = np.arange(P)[:, None]
    cc = np.arange(896)[None, :]
    maskT = (cc >= kk + 384).astype(np.float32)
    return {
        "xT": xT,
        "wqT": wqT,
        "wkT": wkT,
        "wvT": wvT,
        "wlinT": wlinT,
        "wpreT": wpreT,
        "woT": woT,
        "gamma128": gamma128,
        "beta128": beta128,
        "maskT": maskT,
    }


def _run(inputs, trace=False, trace_kwargs=None):
    from concourse.bass_utils import run_bass_kernel_spmd

    if "nc" not in _CACHED:
        _CACHED["nc"] = _build_nc()
    nc = _CACHED["nc"]
    in_maps = [_core_inputs(inputs, c) for c in range(8)]
    res = run_bass_kernel_spmd(
        nc, in_maps, core_ids=list(range(8)), trace=trace,
        **(trace_kwargs or {}),
    )
    x = np.asarray(inputs["x"], dtype=np.float32)
    out = np.empty((B, L, D), dtype=np.float32)
    for b in range(B):
        out[b] = x[b] + res.results[2 * b]["out"] + res.results[2 * b + 1]["out"]
    return out, res


def kernel(**inputs) -> np.ndarray:
    out, _ = _run(inputs, trace=False)
    return out



# revision 2
# speedup vs baseline: 1.0416x; 1.0416x over previous
"""Trainium2 Bass kernel v2 for nn_Block_56126632624726 (dense transformer block).

Reference computation (fp32, B=4, L=2048, D=1024, H=8 heads, hd=128):
    h = LayerNorm(x) * gamma + beta
    [q,k,v,lin,pre] = h @ w_qkv.T            (5*D outputs)
    attn = causal p-softmax attention (p=2)
    branch = [lin * gelu(pre), attn]
    out = x + branch @ w_out.T

Sharding: 8 cores = 4 batches (data parallel) x 2 tensor-parallel halves.
Core j in {0,1} of a batch owns heads 4j..4j+3 plus lin/pre cols
512j..512j+512 and the matching w_out input columns; host sums the two
partial outputs per batch and adds the residual (no device collectives).

v2 design (vs the fp32r v1):
  - fp16 operands everywhere on the PE (1 cyc/row vs ~1.6 for fp32r),
    fp32 PSUM accumulation. gamma folded into w_qkv host-side; beta enters
    as per-partition biases during PSUM evacuation.
  - All weights SBUF-resident (loaded once, fp16): no per-quarter DMA.
  - LN: token sums via ones-matmuls; rsqrt via Act Sqrt + DVE
    reciprocal_approx_fast; per-token scale/shift broadcast across
    partitions with a PE ones-broadcast (no DRAM bounce).
  - Attention per (quarter J, head h) with f' = exp(s*scale - 8*ln2)
    (fp16-safe rescale; cancels in the p=2 normalization), diagonal tiles
    trimmed to the causal query range, r accumulated for all 4 heads of a
    quarter in one [4,512] PSUM tile via an indicator lhsT, o evacuated
    UNNORMALIZED (scaled by 1/4) and normalized at the end from
    cr = rsqrt(r) with a PE broadcast of 4*cr.
  - Out-projection from branchT = [gT; attnT] with fp16 w_out slice,
    fp16 partial output (host upcasts and adds the residual).
"""

import numpy as np

B = 4
L = 2048
D = 1024
P = 128
KC = D // P  # 8 dim chunks
NQ = 4
TQ = L // NQ  # 512
NT = L // P  # 16 token tiles
HL = 4  # heads per core
HD = 128
SCALE = float(HD) ** -0.5
EXPB = -4.0 * float(np.log(2.0))  # exp bias: f' = f * 2^-4 (fp16 range safety)
EPS = 1e-5

_CACHED = {}


def _install_tile_drain_patch(tile, mybir):
    """walrus limits sem waits per SP CTRL instruction to 1; split the
    TileContext final drain's waits across sequential drain instructions."""
    from concourse.vector_clock import ScopedClock

    if getattr(tile.TileContext, "_drain_patched", False):
        return

    def _patched(self, tick_clock, wait_clock):
        drain_inst = self.nc.sync.drain()
        wait_clock.add_sem_waits(
            drain_inst.ins, ScopedClock({None: tick_clock.global_clock})
        )
        si = drain_inst.ins.sync_info
        waits = list(si.on_wait or []) if si else []
        if len(waits) > 1:
            si.on_wait = waits[:1]
            for w in waits[1:]:
                d2 = self.nc.sync.drain()
                d2.ins.sync_info = mybir.SyncInfo(on_wait=[w], on_update=[])
        self.nc.all_engine_barrier()
        popped = self.nc._tile_sem_poison_stack.pop()
        assert popped is self._sem_poison
        self.nc.clear_and_free_semaphores(list(self.sems.allocated().values()))
        self.nc.all_engine_barrier()

    tile.TileContext._drain_and_barrier = _patched
    tile.TileContext._drain_patched = True


def _split_multi_waits(nc, mybir):
    """This walrus build supports at most ONE sync-wait per instruction.
    Move extra waits onto same-engine nops inserted before the instruction."""
    eng_builder = {
        mybir.EngineType.PE: nc.tensor,
        mybir.EngineType.DVE: nc.vector,
        mybir.EngineType.Activation: nc.scalar,
        mybir.EngineType.SP: nc.sync,
        mybir.EngineType.Pool: nc.gpsimd,
    }

    def make_nop(engine):
        bi = eng_builder[engine].nop(nofuse=True)
        inst = bi.ins
        nc.cur_bb.bb.instructions.remove(inst)
        return inst

    for f in nc.m.functions:
        for bb in f.blocks:
            insts = bb.instructions
            rebuilt = []
            changed = False
            for inst in list(insts):
                si = inst.sync_info
                waits = list(si.on_wait or []) if si else []
                if len(waits) > 1:
                    changed = True
                    for w in waits[:-1]:
                        nop = make_nop(inst.engine)
                        nop.sync_info = mybir.SyncInfo(on_wait=[w], on_update=[])
                        rebuilt.append(nop)
                    si.on_wait = waits[-1:]
                rebuilt.append(inst)
            if changed:
                insts.clear()
                insts.extend(rebuilt)


def _build_nc():
    import concourse.bass as bass
    import concourse.tile as tile
    from concourse import mybir

    _install_tile_drain_patch(tile, mybir)

    f32 = mybir.dt.float32
    f16 = mybir.dt.float16
    bf16 = mybir.dt.bfloat16
    AF = mybir.ActivationFunctionType
    OP = mybir.AluOpType

    nc = bass.Bass()

    xT = nc.declare_dram_parameter("xT16", [D, L], f16, isOutput=False)
    wq = nc.declare_dram_parameter("wq16", [D, TQ], f16, isOutput=False)
    wk = nc.declare_dram_parameter("wk16", [D, TQ], f16, isOutput=False)
    wv = nc.declare_dram_parameter("wv16", [D, TQ], f16, isOutput=False)
    wl = nc.declare_dram_parameter("wl16", [D, TQ], f16, isOutput=False)
    wp = nc.declare_dram_parameter("wp16", [D, TQ], f16, isOutput=False)
    wo = nc.declare_dram_parameter("wo16", [D, D], f16, isOutput=False)
    bqk = nc.declare_dram_parameter("bqk", [P, 8], f32, isOutput=False)
    blp = nc.declare_dram_parameter("blp", [P, 8], f32, isOutput=False)
    bv = nc.declare_dram_parameter("bv16", [1, TQ], f16, isOutput=False)
    maskT = nc.declare_dram_parameter("mask16", [P, 896], f16, isOutput=False)
    out = nc.declare_dram_parameter("out16", [L, D], f16, isOutput=True)

    x_r = xT.rearrange("(o p) t -> p o t", p=P)  # [128, 8, 2048]
    wq_r = wq.rearrange("(o p) f -> p o f", p=P)  # [128, 8, 512]
    wk_r = wk.rearrange("(o p) f -> p o f", p=P)
    wv_r = wv.rearrange("(o p) f -> p o f", p=P)
    wl_r = wl.rearrange("(o p) f -> p o f", p=P)
    wp_r = wp.rearrange("(o p) f -> p o f", p=P)
    wo_r = wo.rearrange("(o p) f -> p o f", p=P)  # [128, 8, 1024]

    with tile.TileContext(nc) as tc:
        with tc.tile_pool(name="persist", bufs=1) as ps_pool:
            # ---- persistent SBUF residents ----
            xh = ps_pool.tile([P, KC, L], f16)  # x, normalized in place
            for _q in range(NQ):
                _qs = slice(TQ * _q, TQ * _q + TQ)
                nc.sync.dma_start(out=xh[:, :, _qs], in_=x_r[:, :, _qs])
            wq_s = ps_pool.tile([P, KC, TQ], f16)
            nc.sync.dma_start(out=wq_s[:], in_=wq_r[:])
            wk_s = ps_pool.tile([P, KC, TQ], f16)
            nc.sync.dma_start(out=wk_s[:], in_=wk_r[:])
            wv_s = ps_pool.tile([P, KC, TQ], f16)
            nc.sync.dma_start(out=wv_s[:], in_=wv_r[:])
            wl_s = ps_pool.tile([P, KC, TQ], f16)
            nc.sync.dma_start(out=wl_s[:], in_=wl_r[:])
            wp_s = ps_pool.tile([P, KC, TQ], f16)
            nc.sync.dma_start(out=wp_s[:], in_=wp_r[:])
            wo_s = ps_pool.tile([P, KC, D], f16)
            nc.sync.dma_start(out=wo_s[:], in_=wo_r[:])
            masks = ps_pool.tile([P, 896], f16)
            nc.sync.dma_start(out=masks[:], in_=maskT[:])
            bqk_s = ps_pool.tile([P, 8], f32)
            nc.sync.dma_start(out=bqk_s[:], in_=bqk[:])
            blp_s = ps_pool.tile([P, 8], f32)
            nc.sync.dma_start(out=blp_s[:], in_=blp[:])
            bv_s = ps_pool.tile([1, TQ], f16)
            nc.sync.dma_start(out=bv_s[:], in_=bv[:])

            QT = ps_pool.tile([P, HL, L], f16)
            KT = ps_pool.tile([P, HL, L], f16)
            V = ps_pool.tile([P, NT, TQ], f16)  # token-major
            gT = ps_pool.tile([P, HL, L], f16)
            attnT = ps_pool.tile([P, HL, L], f16)  # o'/4, normalized in place

            onesc = ps_pool.tile([P, 1], f16)
            nc.vector.memset(onesc, 1.0)
            onesb = ps_pool.tile([P, 1], bf16)
            nc.vector.memset(onesb, 1.0)
            onesr = ps_pool.tile([1, P], f16)
            nc.vector.memset(onesr, 1.0)
            epst = ps_pool.tile([1, 1], f32)
            nc.vector.memset(epst, EPS)
            expb = ps_pool.tile([P, 1], f32)
            nc.vector.memset(expb, EXPB)

            # =========== Phase 1: LN + projections ===========
            with (
                tc.tile_pool(name="sq", bufs=3) as sqp,
                tc.tile_pool(name="rows", bufs=2) as rows_p,
                tc.tile_pool(name="bc", bufs=2) as bcp,
                tc.tile_pool(name="gel", bufs=2) as gelp,
                tc.tile_pool(name="lin", bufs=2) as linp,
                tc.tile_pool(name="rps", bufs=2, space="PSUM") as rps,
                tc.tile_pool(name="bps", bufs=1, space="PSUM") as bps,
                tc.tile_pool(name="pps", bufs=2, space="PSUM") as pps,
            ):

                def stats(q):
                    qsl = slice(TQ * q, TQ * q + TQ)
                    s1 = rps.tile([1, TQ], f32, tag="s1", name=f"s1_{q}")
                    s2 = rps.tile([1, TQ], f32, tag="s2", name=f"s2_{q}")
                    for k in range(KC):
                        nc.tensor.matmul(
                            s1, lhsT=onesc[:], rhs=xh[:, k, qsl],
                            start=(k == 0), stop=(k == KC - 1),
                        )
                    for k in range(KC):
                        x2 = sqp.tile([P, TQ], f16, tag="x2")
                        nc.vector.tensor_mul(
                            out=x2[:], in0=xh[:, k, qsl], in1=xh[:, k, qsl]
                        )
                        nc.tensor.matmul(
                            s2, lhsT=onesc[:], rhs=x2[:],
                            start=(k == 0), stop=(k == KC - 1),
                        )
                    return s1, s2

                def rows_and_bcast(q, s1, s2):
                    # mu/var/inv rows + fp16 cast + PE broadcast
                    mu = rows_p.tile([1, TQ], f32, tag="mu")
                    nc.scalar.mul(out=mu[:], in_=s1, mul=1.0 / D)
                    m2 = rows_p.tile([1, TQ], f32, tag="m2")
                    nc.scalar.mul(out=m2[:], in_=s2, mul=1.0 / D)
                    var = rows_p.tile([1, TQ], f32, tag="var")
                    nc.vector.tensor_mul(out=var[:], in0=mu[:], in1=mu[:])
                    nc.vector.tensor_tensor(
                        out=var[:], in0=m2[:], in1=var[:], op=OP.subtract
                    )
                    # inv = rsqrt(var+eps) via ln->exp (both in one act table)
                    lnv = rows_p.tile([1, TQ], f32, tag="lnv")
                    nc.scalar.activation(
                        out=lnv[:], in_=var[:], func=AF.Ln, bias=epst[:]
                    )
                    inv16 = rows_p.tile([1, TQ], f16, tag="inv16")
                    nc.scalar.activation(
                        out=inv16[:], in_=lnv[:], func=AF.Exp, scale=-0.5
                    )
                    nmu = rows_p.tile([1, TQ], f32, tag="nmu")
                    nc.scalar.mul(out=nmu[:], in_=s1, mul=-1.0 / D)
                    ninv16 = rows_p.tile([1, TQ], f16, tag="ninv16")
                    nc.vector.tensor_mul(out=ninv16[:], in0=nmu[:], in1=inv16[:])
                    inb_ps = bps.tile([P, TQ], f32, tag="inb", name=f"inb{q}")
                    nc.tensor.matmul(
                        inb_ps, lhsT=onesr[:], rhs=inv16[:], start=True, stop=True
                    )
                    nnb_ps = bps.tile([P, TQ], f32, tag="nnb", name=f"nnb{q}")
                    nc.tensor.matmul(
                        nnb_ps, lhsT=onesr[:], rhs=ninv16[:], start=True, stop=True
                    )
                    invb = bcp.tile([P, TQ], f16, tag="invb")
                    nc.scalar.copy(out=invb[:], in_=inb_ps)
                    ninvb = bcp.tile([P, TQ], f16, tag="ninvb")
                    nc.vector.tensor_copy(out=ninvb[:], in_=nnb_ps)
                    return invb, ninvb

                def normalize(q, invb, ninvb):
                    qsl = slice(TQ * q, TQ * q + TQ)
                    for k in range(KC):
                        nc.vector.tensor_mul(
                            out=xh[:, k, qsl], in0=xh[:, k, qsl], in1=invb[:]
                        )
                        nc.vector.tensor_add(
                            out=xh[:, k, qsl], in0=xh[:, k, qsl], in1=ninvb[:]
                        )

                def proj(q):
                    qsl = slice(TQ * q, TQ * q + TQ)
                    # q/k heads -> QT/KT (dim-major), Act Identity evac w/ bias
                    for dst, wsb, bcol0 in ((QT, wq_s, 0), (KT, wk_s, 4)):
                        for m in range(HL):
                            ps = pps.tile([P, TQ], f32, tag="mm")
                            for k in range(KC):
                                nc.tensor.matmul(
                                    ps, lhsT=wsb[:, k, P * m : P * m + P],
                                    rhs=xh[:, k, qsl],
                                    start=(k == 0), stop=(k == KC - 1),
                                )
                            nc.scalar.activation(
                                out=dst[:, m, qsl], in_=ps, func=AF.Identity,
                                bias=bqk_s[:, bcol0 + m : bcol0 + m + 1],
                            )
                    # gT = (lin+b) * gelu(pre+b)
                    for m in range(HL):
                        psp = pps.tile([P, TQ], f32, tag="mm")
                        for k in range(KC):
                            nc.tensor.matmul(
                                psp, lhsT=wp_s[:, k, P * m : P * m + P],
                                rhs=xh[:, k, qsl],
                                start=(k == 0), stop=(k == KC - 1),
                            )
                        gel = gelp.tile([P, TQ], f16, tag="gel")
                        nc.scalar.activation(
                            out=gel[:], in_=psp, func=AF.Gelu,
                            bias=blp_s[:, 4 + m : 4 + m + 1],
                        )
                        psl = pps.tile([P, TQ], f32, tag="mm")
                        for k in range(KC):
                            nc.tensor.matmul(
                                psl, lhsT=wl_s[:, k, P * m : P * m + P],
                                rhs=xh[:, k, qsl],
                                start=(k == 0), stop=(k == KC - 1),
                            )
                        lnb = linp.tile([P, TQ], f16, tag="lnb")
                        nc.vector.tensor_scalar(
                            out=lnb[:], in0=psl,
                            scalar1=blp_s[:, m : m + 1], scalar2=None,
                            op0=OP.add,
                        )
                        nc.vector.tensor_mul(
                            out=gT[:, m, qsl], in0=lnb[:], in1=gel[:]
                        )
                    # V (token-major) via swapped matmul, bias row via ones-matmul
                    for i in range(NQ):
                        vps = pps.tile([P, TQ], f32, tag="mm")
                        nc.tensor.matmul(
                            vps, lhsT=onesr[:], rhs=bv_s[:], start=True, stop=False
                        )
                        tsl = slice(TQ * q + P * i, TQ * q + P * i + P)
                        for k in range(KC):
                            nc.tensor.matmul(
                                vps, lhsT=xh[:, k, tsl], rhs=wv_s[:, k, :],
                                start=False, stop=(k == KC - 1),
                            )
                        nc.vector.tensor_copy(out=V[:, NQ * q + i, :], in_=vps)

                s1, s2 = stats(0)
                for q in range(NQ):
                    invb, ninvb = rows_and_bcast(q, s1, s2)
                    normalize(q, invb, ninvb)
                    if q < NQ - 1:
                        s1, s2 = stats(q + 1)
                    proj(q)

            # =========== Phase 2: attention ===========
            # attnT holds o'/4, normalized per quarter with cr = rsqrt(r')
            # (ln->exp on the act engine; exp/ln share a table so there is
            # no act-table thrash). The missing 4x is folded into w_out
            # host-side. cr rows are broadcast across partitions with a
            # DRAM-bounce DMA.
            import concourse.bass as _b

            QS = (0, P, 2 * P, 2 * P)  # diagonal-tile query range starts
            with (
                tc.tile_pool(name="fp", bufs=4) as fp,
                tc.tile_pool(name="ep", bufs=3) as ep,
                tc.tile_pool(name="rbp", bufs=1) as rbp,
                tc.tile_pool(name="crp", bufs=1) as crp,
                tc.tile_pool(name="cbt", bufs=2) as cbtp,
                tc.tile_pool(name="rowd", bufs=2, space="DRAM") as rowd,
                tc.tile_pool(name="stps", bufs=3, space="PSUM") as stps,
                tc.tile_pool(name="ops", bufs=2, space="PSUM") as ops,
                tc.tile_pool(name="arps", bufs=2, space="PSUM") as arps,
            ):
                for J in range(NQ):
                    jsl = slice(TQ * J, TQ * J + TQ)
                    nt = 4 * J + 4
                    rbufJ = rbp.tile([1, HL * TQ], f32, tag="rb", name=f"rb{J}")
                    for h in range(HL):
                        o_ps = ops.tile([P, TQ], f32, tag="o")
                        r_ps = arps.tile([1, TQ], f32, tag="r")

                        def qext(t):
                            d = t - 4 * J
                            return (QS[d], TQ - QS[d]) if d >= 0 else (0, TQ)

                        def scores(t):
                            qs, ext = qext(t)
                            st = stps.tile([P, TQ], f32, tag="st")
                            nc.tensor.matmul(
                                st[:, qs : qs + ext],
                                lhsT=KT[:, h, P * t : P * t + P],
                                rhs=QT[:, h, TQ * J + qs : TQ * J + qs + ext],
                                start=True, stop=True,
                            )
                            f = fp.tile([P, TQ], f16, tag="f")
                            nc.scalar.activation(
                                out=f[:, :ext], in_=st[:, qs : qs + ext],
                                func=AF.Exp, scale=SCALE, bias=expb[:],
                            )
                            d = t - 4 * J
                            if d >= 0:
                                off = 384 - (P * d - qs)
                                nc.vector.tensor_mul(
                                    out=f[:, :ext], in0=f[:, :ext],
                                    in1=masks[:, off : off + ext],
                                )
                            e = ep.tile([P, TQ], bf16, tag="e")
                            nc.vector.tensor_mul(
                                out=e[:, :ext], in0=f[:, :ext], in1=f[:, :ext]
                            )
                            return f, e

                        def accum(t, f, e):
                            qs, ext = qext(t)
                            nc.tensor.matmul(
                                r_ps[:, qs : qs + ext],
                                lhsT=onesb[:],
                                rhs=e[:, :ext],
                                start=(t == 0), stop=(t == nt - 1),
                            )
                            nc.tensor.matmul(
                                o_ps[:, qs : qs + ext],
                                lhsT=V[:, t, P * h : P * h + P],
                                rhs=f[:, :ext],
                                start=(t == 0), stop=(t == nt - 1),
                            )

                        # software pipeline: scores(t+1) issues before accum(t)
                        fe = scores(0)
                        for t in range(nt):
                            fe_next = scores(t + 1) if t + 1 < nt else None
                            accum(t, *fe)
                            fe = fe_next
                        # unnormalized o'/4 -> attnT; r row -> rbufJ
                        nc.vector.tensor_scalar(
                            out=attnT[:, h, jsl], in0=o_ps,
                            scalar1=0.25, scalar2=None, op0=OP.mult,
                        )
                        nc.vector.tensor_copy(
                            out=rbufJ[:, TQ * h : TQ * h + TQ], in_=r_ps
                        )
                    # cr = rsqrt(r') for the 4 heads of this quarter
                    lnr = rbp.tile([1, HL * TQ], f32, tag="ln", name=f"ln{J}")
                    nc.scalar.activation(out=lnr[:], in_=rbufJ[:], func=AF.Ln)
                    crJ = crp.tile([1, HL * TQ], f16, tag="cr", name=f"cr{J}")
                    nc.scalar.activation(
                        out=crJ[:], in_=lnr[:], func=AF.Exp, scale=-0.5
                    )
                    # broadcast each head's cr row and normalize attnT
                    for h in range(HL):
                        rd = rowd.tile([1, TQ], f16, tag="rd")
                        nc.sync.dma_start(
                            out=rd, in_=crJ[:, TQ * h : TQ * h + TQ]
                        )
                        bap = _b.AP(
                            tensor=rd.tensor, offset=rd.offset,
                            ap=[[0, P], [1, TQ]],
                        )
                        cbt = cbtp.tile([P, TQ], f16, tag="cb")
                        nc.sync.dma_start(out=cbt[:], in_=bap)
                        nc.vector.tensor_mul(
                            out=attnT[:, h, jsl], in0=attnT[:, h, jsl],
                            in1=cbt[:],
                        )

            # =========== Phase 3: out projection ===========
            with (
                tc.tile_pool(name="obuf", bufs=4) as obuf,
                tc.tile_pool(name="p3", bufs=4, space="PSUM") as p3ps,
            ):
                for i in range(NT):
                    for n in range(2):
                        ps3 = p3ps.tile([P, TQ], f32, tag="o3")
                        for c in range(KC):
                            src = gT if c < HL else attnT
                            nc.tensor.matmul(
                                ps3,
                                lhsT=src[:, c % HL, P * i : P * i + P],
                                rhs=wo_s[:, c, TQ * n : TQ * n + TQ],
                                start=(c == 0), stop=(c == KC - 1),
                            )
                        ot = obuf.tile([P, TQ], f16, tag="ot")
                        if n == 0:
                            nc.vector.tensor_copy(out=ot[:], in_=ps3)
                        else:
                            nc.scalar.copy(out=ot[:], in_=ps3)
                        nc.sync.dma_start(
                            out=out[P * i : P * i + P, TQ * n : TQ * n + TQ],
                            in_=ot[:],
                        )

    _split_multi_waits(nc, mybir)
    return nc


def _core_inputs(inputs, core):
    """Per-core input map: host-side sharding, fp16 casts, gamma folding."""
    x = np.asarray(inputs["x"], dtype=np.float32)
    gamma = np.asarray(inputs["gamma"], dtype=np.float32)
    beta = np.asarray(inputs["beta"], dtype=np.float32)
    w_qkv = np.asarray(inputs["w_qkv"], dtype=np.float32)
    w_out = np.asarray(inputs["w_out"], dtype=np.float32)

    b, j = core // 2, core % 2
    sl = slice(512 * j, 512 * j + 512)

    def wslice(base):
        wsub = w_qkv[base : base + D][sl]  # [512 out, 1024 in]
        wg = wsub * gamma[None, :]
        bias = wsub @ beta  # [512]
        return np.ascontiguousarray(wg.T).astype(np.float16), bias.astype(np.float32)

    wq16, bq = wslice(0)
    wk16, bk = wslice(D)
    wv16, bvr = wslice(2 * D)
    wl16, bl = wslice(3 * D)
    wp16, bp = wslice(4 * D)

    bqk = np.stack(
        [bq[128 * t : 128 * t + 128] for t in range(4)]
        + [bk[128 * t : 128 * t + 128] for t in range(4)],
        axis=1,
    )
    blp = np.stack(
        [bl[128 * t : 128 * t + 128] for t in range(4)]
        + [bp[128 * t : 128 * t + 128] for t in range(4)],
        axis=1,
    )

    cols = np.r_[512 * j : 512 * j + 512, D + 512 * j : D + 512 * j + 512]
    wo_sel = w_out[:, cols].copy()
    wo_sel[:, 512:] *= 4.0  # attnT stores o'/4; fold the 4x back here
    wo16 = np.ascontiguousarray(wo_sel.T).astype(np.float16)

    kk = np.arange(P)[:, None]
    cc = np.arange(896)[None, :]
    mask16 = (cc >= kk + 384).astype(np.float16)

    return {
        "xT16": np.ascontiguousarray(x[b].T).astype(np.float16),
        "wq16": wq16,
        "wk16": wk16,
        "wv16": wv16,
        "wl16": wl16,
        "wp16": wp16,
        "wo16": wo16,
        "bqk": np.ascontiguousarray(bqk),
        "blp": np.ascontiguousarray(blp),
        "bv16": bvr.astype(np.float16)[None, :],
        "mask16": mask16,
    }


def _run(inputs, trace=False, trace_kwargs=None):
    from concourse.bass_utils import run_bass_kernel_spmd

    if "nc" not in _CACHED:
        _CACHED["nc"] = _build_nc()
    nc = _CACHED["nc"]
    in_maps = [_core_inputs(inputs, c) for c in range(8)]
    res = run_bass_kernel_spmd(
        nc, in_maps, core_ids=list(range(8)), trace=trace,
        **(trace_kwargs or {}),
    )
    x = np.asarray(inputs["x"], dtype=np.float32)
    out = np.empty((B, L, D), dtype=np.float32)
    for b in range(B):
        out[b] = x[b] + (
            res.results[2 * b]["out16"].astype(np.float32)
            + res.results[2 * b + 1]["out16"].astype(np.float32)
        )
    return out, res


def kernel(**inputs) -> np.ndarray:
    out, _ = _run(inputs, trace=False)
    return out


# revision 3
# speedup vs baseline: 1.0730x; 1.0301x over previous
"""Trainium2 Bass kernel v2 for nn_Block_56126632624726 (dense transformer block).

Reference computation (fp32, B=4, L=2048, D=1024, H=8 heads, hd=128):
    h = LayerNorm(x) * gamma + beta
    [q,k,v,lin,pre] = h @ w_qkv.T            (5*D outputs)
    attn = causal p-softmax attention (p=2)
    branch = [lin * gelu(pre), attn]
    out = x + branch @ w_out.T

Sharding: 8 cores = 4 batches (data parallel) x 2 tensor-parallel halves.
Core j in {0,1} of a batch owns heads 4j..4j+3 plus lin/pre cols
512j..512j+512 and the matching w_out input columns; host sums the two
partial outputs per batch and adds the residual (no device collectives).

v2 design (vs the fp32r v1):
  - fp16 operands everywhere on the PE (1 cyc/row vs ~1.6 for fp32r),
    fp32 PSUM accumulation. gamma folded into w_qkv host-side; beta enters
    as per-partition biases during PSUM evacuation.
  - All weights SBUF-resident (loaded once, fp16): no per-quarter DMA.
  - LN: token sums via ones-matmuls; rsqrt via Act Sqrt + DVE
    reciprocal_approx_fast; per-token scale/shift broadcast across
    partitions with a PE ones-broadcast (no DRAM bounce).
  - Attention per (quarter J, head h) with f' = exp(s*scale - 8*ln2)
    (fp16-safe rescale; cancels in the p=2 normalization), diagonal tiles
    trimmed to the causal query range, r accumulated for all 4 heads of a
    quarter in one [4,512] PSUM tile via an indicator lhsT, o evacuated
    UNNORMALIZED (scaled by 1/4) and normalized at the end from
    cr = rsqrt(r) with a PE broadcast of 4*cr.
  - Out-projection from branchT = [gT; attnT] with fp16 w_out slice,
    fp16 partial output (host upcasts and adds the residual).
"""

import numpy as np

B = 4
L = 2048
D = 1024
P = 128
KC = D // P  # 8 dim chunks
NQ = 4
TQ = L // NQ  # 512
NT = L // P  # 16 token tiles
HL = 4  # heads per core
HD = 128
SCALE = float(HD) ** -0.5
EXPB = -4.0 * float(np.log(2.0))  # exp bias: f' = f * 2^-4 (fp16 range safety)
EPS = 1e-5

_CACHED = {}


def _install_tile_drain_patch(tile, mybir):
    """walrus limits sem waits per SP CTRL instruction to 1; split the
    TileContext final drain's waits across sequential drain instructions."""
    from concourse.vector_clock import ScopedClock

    if getattr(tile.TileContext, "_drain_patched", False):
        return

    def _patched(self, tick_clock, wait_clock):
        drain_inst = self.nc.sync.drain()
        wait_clock.add_sem_waits(
            drain_inst.ins, ScopedClock({None: tick_clock.global_clock})
        )
        si = drain_inst.ins.sync_info
        waits = list(si.on_wait or []) if si else []
        if len(waits) > 1:
            si.on_wait = waits[:1]
            for w in waits[1:]:
                d2 = self.nc.sync.drain()
                d2.ins.sync_info = mybir.SyncInfo(on_wait=[w], on_update=[])
        self.nc.all_engine_barrier()
        popped = self.nc._tile_sem_poison_stack.pop()
        assert popped is self._sem_poison
        self.nc.clear_and_free_semaphores(list(self.sems.allocated().values()))
        self.nc.all_engine_barrier()

    tile.TileContext._drain_and_barrier = _patched
    tile.TileContext._drain_patched = True


def _split_multi_waits(nc, mybir):
    """This walrus build supports at most ONE sync-wait per instruction.
    Move extra waits onto same-engine nops inserted before the instruction."""
    eng_builder = {
        mybir.EngineType.PE: nc.tensor,
        mybir.EngineType.DVE: nc.vector,
        mybir.EngineType.Activation: nc.scalar,
        mybir.EngineType.SP: nc.sync,
        mybir.EngineType.Pool: nc.gpsimd,
    }

    def make_nop(engine):
        bi = eng_builder[engine].nop(nofuse=True)
        inst = bi.ins
        nc.cur_bb.bb.instructions.remove(inst)
        return inst

    for f in nc.m.functions:
        for bb in f.blocks:
            insts = bb.instructions
            rebuilt = []
            changed = False
            for inst in list(insts):
                si = inst.sync_info
                waits = list(si.on_wait or []) if si else []
                if len(waits) > 1:
                    changed = True
                    for w in waits[:-1]:
                        nop = make_nop(inst.engine)
                        nop.sync_info = mybir.SyncInfo(on_wait=[w], on_update=[])
                        rebuilt.append(nop)
                    si.on_wait = waits[-1:]
                rebuilt.append(inst)
            if changed:
                insts.clear()
                insts.extend(rebuilt)


def _build_nc(beta_zero=False):
    import concourse.bass as bass
    import concourse.tile as tile
    from concourse import mybir

    _install_tile_drain_patch(tile, mybir)

    f32 = mybir.dt.float32
    f16 = mybir.dt.float16
    bf16 = mybir.dt.bfloat16
    AF = mybir.ActivationFunctionType
    OP = mybir.AluOpType

    nc = bass.Bass()

    xT = nc.declare_dram_parameter("xT16", [D, L], f16, isOutput=False)
    wq = nc.declare_dram_parameter("wq16", [D, TQ], f16, isOutput=False)
    wk = nc.declare_dram_parameter("wk16", [D, TQ], f16, isOutput=False)
    wv = nc.declare_dram_parameter("wv16", [D, TQ], f16, isOutput=False)
    wl = nc.declare_dram_parameter("wl16", [D, TQ], f16, isOutput=False)
    wp = nc.declare_dram_parameter("wp16", [D, TQ], f16, isOutput=False)
    wo = nc.declare_dram_parameter("wo16", [D, D], f16, isOutput=False)
    bqk = nc.declare_dram_parameter("bqk", [P, 8], f32, isOutput=False)
    blp = nc.declare_dram_parameter("blp", [P, 8], f32, isOutput=False)
    bv = nc.declare_dram_parameter("bv16", [1, TQ], f16, isOutput=False)
    maskT = nc.declare_dram_parameter("mask16", [P, 896], f16, isOutput=False)
    out = nc.declare_dram_parameter("out16", [L, D], f16, isOutput=True)

    x_r = xT.rearrange("(o p) t -> p o t", p=P)  # [128, 8, 2048]
    wq_r = wq.rearrange("(o p) f -> p o f", p=P)  # [128, 8, 512]
    wk_r = wk.rearrange("(o p) f -> p o f", p=P)
    wv_r = wv.rearrange("(o p) f -> p o f", p=P)
    wl_r = wl.rearrange("(o p) f -> p o f", p=P)
    wp_r = wp.rearrange("(o p) f -> p o f", p=P)
    wo_r = wo.rearrange("(o p) f -> p o f", p=P)  # [128, 8, 1024]

    with tile.TileContext(nc) as tc:
        with tc.tile_pool(name="persist", bufs=1) as ps_pool:
            # ---- persistent SBUF residents ----
            xh = ps_pool.tile([P, KC, L], f16)  # x, normalized in place
            for _q in range(NQ):
                _qs = slice(TQ * _q, TQ * _q + TQ)
                nc.sync.dma_start(out=xh[:, 0:4, _qs], in_=x_r[:, 0:4, _qs])
                nc.sync.dma_start(out=xh[:, 4:8, _qs], in_=x_r[:, 4:8, _qs])
            wq_s = ps_pool.tile([P, KC, TQ], f16)
            nc.sync.dma_start(out=wq_s[:], in_=wq_r[:])
            wk_s = ps_pool.tile([P, KC, TQ], f16)
            nc.sync.dma_start(out=wk_s[:], in_=wk_r[:])
            wv_s = ps_pool.tile([P, KC, TQ], f16)
            nc.sync.dma_start(out=wv_s[:], in_=wv_r[:])
            wl_s = ps_pool.tile([P, KC, TQ], f16)
            nc.sync.dma_start(out=wl_s[:], in_=wl_r[:])
            wp_s = ps_pool.tile([P, KC, TQ], f16)
            nc.sync.dma_start(out=wp_s[:], in_=wp_r[:])
            wo_s = ps_pool.tile([P, KC, D], f16)
            nc.sync.dma_start(out=wo_s[:], in_=wo_r[:])
            masks = ps_pool.tile([P, 896], f16)
            nc.sync.dma_start(out=masks[:], in_=maskT[:])
            bqk_s = ps_pool.tile([P, 8], f32)
            nc.sync.dma_start(out=bqk_s[:], in_=bqk[:])
            blp_s = ps_pool.tile([P, 8], f32)
            nc.sync.dma_start(out=blp_s[:], in_=blp[:])
            bv_s = ps_pool.tile([1, TQ], f16)
            nc.sync.dma_start(out=bv_s[:], in_=bv[:])

            QT = ps_pool.tile([P, HL, L], f16)
            KT = ps_pool.tile([P, HL, L], f16)
            V = ps_pool.tile([P, NT, TQ], f16)  # token-major
            gT = ps_pool.tile([P, HL, L], f16)
            attnT = ps_pool.tile([P, HL, L], f16)  # o'/4, normalized in place

            onesc = ps_pool.tile([P, 1], f16)
            nc.vector.memset(onesc, 1.0)
            onesb = ps_pool.tile([P, 1], bf16)
            nc.vector.memset(onesb, 1.0)
            onesr = ps_pool.tile([1, P], f16)
            nc.vector.memset(onesr, 1.0)
            epst = ps_pool.tile([1, 1], f32)
            nc.vector.memset(epst, EPS)
            expb = ps_pool.tile([P, 1], f32)
            nc.vector.memset(expb, EXPB)

            # =========== Phase 1: LN + projections ===========
            with (
                tc.tile_pool(name="sq", bufs=2) as sqp,
                tc.tile_pool(name="rows", bufs=2) as rows_p,
                tc.tile_pool(name="bc", bufs=3) as bcp,
                tc.tile_pool(name="gel", bufs=2) as gelp,
                tc.tile_pool(name="lin", bufs=2) as linp,
                tc.tile_pool(name="rps", bufs=2, space="PSUM") as rps,
                tc.tile_pool(name="bps", bufs=1, space="PSUM") as bps,
                tc.tile_pool(name="pps", bufs=2, space="PSUM") as pps,
            ):

                def stats(q):
                    qsl = slice(TQ * q, TQ * q + TQ)
                    s1 = rps.tile([1, TQ], f32, tag="s1", name=f"s1_{q}")
                    s2 = rps.tile([1, TQ], f32, tag="s2", name=f"s2_{q}")
                    for k in range(KC):
                        nc.tensor.matmul(
                            s1, lhsT=onesc[:], rhs=xh[:, k, qsl],
                            start=(k == 0), stop=(k == KC - 1),
                        )
                    for k in range(KC):
                        x2 = sqp.tile([P, TQ], f16, tag="x2")
                        nc.vector.tensor_mul(
                            out=x2[:], in0=xh[:, k, qsl], in1=xh[:, k, qsl]
                        )
                        nc.tensor.matmul(
                            s2, lhsT=onesc[:], rhs=x2[:],
                            start=(k == 0), stop=(k == KC - 1),
                        )
                    return s1, s2

                def rows_math(q, s1, s2):
                    # mu/var/inv rows + fp16 cast (act/dve only, no PE)
                    mu = rows_p.tile([1, TQ], f32, tag="mu")
                    nc.scalar.mul(out=mu[:], in_=s1, mul=1.0 / D)
                    m2 = rows_p.tile([1, TQ], f32, tag="m2")
                    nc.scalar.mul(out=m2[:], in_=s2, mul=1.0 / D)
                    var = rows_p.tile([1, TQ], f32, tag="var")
                    nc.vector.tensor_mul(out=var[:], in0=mu[:], in1=mu[:])
                    nc.vector.tensor_tensor(
                        out=var[:], in0=m2[:], in1=var[:], op=OP.subtract
                    )
                    # inv = rsqrt(var+eps) via ln->exp (both in one act table)
                    lnv = rows_p.tile([1, TQ], f32, tag="lnv")
                    nc.scalar.activation(
                        out=lnv[:], in_=var[:], func=AF.Ln, bias=epst[:]
                    )
                    inv16 = rows_p.tile([1, TQ], f16, tag="inv16", name=f"iv{q}")
                    nc.scalar.activation(
                        out=inv16[:], in_=lnv[:], func=AF.Exp, scale=-0.5
                    )
                    nmu = rows_p.tile([1, TQ], f32, tag="nmu")
                    nc.scalar.mul(out=nmu[:], in_=s1, mul=-1.0 / D)
                    ninv16 = rows_p.tile([1, TQ], f16, tag="ninv16", name=f"nv{q}")
                    nc.vector.tensor_mul(out=ninv16[:], in0=nmu[:], in1=inv16[:])
                    return inv16, ninv16

                def bcast(q, inv16, ninv16):
                    inb_ps = bps.tile([P, TQ], f32, tag="inb", name=f"inb{q}")
                    nc.tensor.matmul(
                        inb_ps, lhsT=onesr[:], rhs=inv16[:], start=True, stop=True
                    )
                    nnb_ps = bps.tile([P, TQ], f32, tag="nnb", name=f"nnb{q}")
                    nc.tensor.matmul(
                        nnb_ps, lhsT=onesr[:], rhs=ninv16[:], start=True, stop=True
                    )
                    invb = bcp.tile([P, TQ], f16, tag="invb", name=f"ib{q}")
                    nc.scalar.copy(out=invb[:], in_=inb_ps)
                    ninvb = bcp.tile([P, TQ], f16, tag="ninvb", name=f"nb{q}")
                    nc.vector.tensor_copy(out=ninvb[:], in_=nnb_ps)
                    return invb, ninvb

                def normalize(q, invb, ninvb):
                    qsl = slice(TQ * q, TQ * q + TQ)
                    for k in range(KC):
                        nc.vector.tensor_mul(
                            out=xh[:, k, qsl], in0=xh[:, k, qsl], in1=invb[:]
                        )
                        nc.vector.tensor_add(
                            out=xh[:, k, qsl], in0=xh[:, k, qsl], in1=ninvb[:]
                        )

                def proj(q):
                    qsl = slice(TQ * q, TQ * q + TQ)
                    # q/k heads -> QT/KT (dim-major), Act Identity evac w/ bias
                    for dst, wsb, bcol0 in ((QT, wq_s, 0), (KT, wk_s, 4)):
                        for m in range(HL):
                            ps = pps.tile([P, TQ], f32, tag="mm")
                            for k in range(KC):
                                nc.tensor.matmul(
                                    ps, lhsT=wsb[:, k, P * m : P * m + P],
                                    rhs=xh[:, k, qsl],
                                    start=(k == 0), stop=(k == KC - 1),
                                )
                            nc.scalar.activation(
                                out=dst[:, m, qsl], in_=ps, func=AF.Identity,
                                bias=bqk_s[:, bcol0 + m : bcol0 + m + 1],
                            )
                    # gT = (lin+b) * gelu(pre+b)
                    for m in range(HL):
                        psp = pps.tile([P, TQ], f32, tag="mm")
                        for k in range(KC):
                            nc.tensor.matmul(
                                psp, lhsT=wp_s[:, k, P * m : P * m + P],
                                rhs=xh[:, k, qsl],
                                start=(k == 0), stop=(k == KC - 1),
                            )
                        gel = gelp.tile([P, TQ], f16, tag="gel")
                        nc.scalar.activation(
                            out=gel[:], in_=psp, func=AF.Gelu,
                            bias=blp_s[:, 4 + m : 4 + m + 1],
                        )
                        psl = pps.tile([P, TQ], f32, tag="mm")
                        for k in range(KC):
                            nc.tensor.matmul(
                                psl, lhsT=wl_s[:, k, P * m : P * m + P],
                                rhs=xh[:, k, qsl],
                                start=(k == 0), stop=(k == KC - 1),
                            )
                        lnb = linp.tile([P, TQ], f16, tag="lnb")
                        nc.vector.tensor_scalar(
                            out=lnb[:], in0=psl,
                            scalar1=blp_s[:, m : m + 1], scalar2=None,
                            op0=OP.add,
                        )
                        nc.vector.tensor_mul(
                            out=gT[:, m, qsl], in0=lnb[:], in1=gel[:]
                        )
                    # V (token-major) via swapped matmul, bias row via ones-matmul
                    for i in range(NQ):
                        vps = pps.tile([P, TQ], f32, tag="mm")
                        if not beta_zero:
                            nc.tensor.matmul(
                                vps, lhsT=onesr[:], rhs=bv_s[:],
                                start=True, stop=False,
                            )
                        tsl = slice(TQ * q + P * i, TQ * q + P * i + P)
                        for k in range(KC):
                            nc.tensor.matmul(
                                vps, lhsT=xh[:, k, tsl], rhs=wv_s[:, k, :],
                                start=(beta_zero and k == 0), stop=(k == KC - 1),
                            )
                        nc.vector.tensor_copy(out=V[:, NQ * q + i, :], in_=vps)

                # LN fully precomputed up front: stats matmuls hide the
                # act/dve row chains, normalizes hide behind later stats/proj.
                st0 = stats(0)
                st1 = stats(1)
                rw0 = rows_math(0, *st0)
                rw1 = rows_math(1, *st1)
                st2 = stats(2)
                st3 = stats(3)
                bc0 = bcast(0, *rw0)
                bc1 = bcast(1, *rw1)
                normalize(0, *bc0)
                normalize(1, *bc1)
                rw2 = rows_math(2, *st2)
                rw3 = rows_math(3, *st3)
                bc2 = bcast(2, *rw2)
                bc3 = bcast(3, *rw3)
                proj(0)
                normalize(2, *bc2)
                proj(1)
                normalize(3, *bc3)
                proj(2)
                proj(3)

            # =========== Phase 2+3: attention fused with out-projection ===========
            # attnT holds o'/4, normalized per quarter with cr = rsqrt(r')
            # (ln->exp on the act engine; exp/ln share a table so there is
            # no act-table thrash). The missing 4x is folded into w_out
            # host-side. cr rows are broadcast across partitions with a
            # DRAM-bounce DMA. Off-diagonal score tiles are processed in
            # pairs (one exp / one square per 1024 columns) to cut act-engine
            # overhead. Out-projection PSUM groups for quarter J-1 are
            # interleaved between head blocks of quarter J to fill PE gaps
            # while the act engine paces the exp chain; their PSUM->SBUF
            # evacuation runs on the otherwise-idle gpsimd engine.
            import concourse.bass as _b

            QS = (0, P, 2 * P, 3 * P)  # diagonal-tile query range starts
            with (
                tc.tile_pool(name="fp", bufs=4) as fp,
                tc.tile_pool(name="ep", bufs=3) as ep,
                tc.tile_pool(name="rbp", bufs=1) as rbp,
                tc.tile_pool(name="crp", bufs=1) as crp,
                tc.tile_pool(name="cbt", bufs=2) as cbtp,
                tc.tile_pool(name="obuf", bufs=3) as obuf,
                tc.tile_pool(name="rowd", bufs=2, space="DRAM") as rowd,
                tc.tile_pool(name="stps", bufs=3, space="PSUM") as stps,
                tc.tile_pool(name="ops", bufs=2, space="PSUM") as ops,
                tc.tile_pool(name="arps", bufs=1, space="PSUM") as arps,
                tc.tile_pool(name="p3ps", bufs=2, space="PSUM") as p3ps,
            ):
                def p3_group(i, n):
                    ps3 = p3ps.tile([P, TQ], f32, tag="o3")
                    for c in range(KC):
                        src3 = gT if c < HL else attnT
                        nc.tensor.matmul(
                            ps3,
                            lhsT=src3[:, c % HL, P * i : P * i + P],
                            rhs=wo_s[:, c, TQ * n : TQ * n + TQ],
                            start=(c == 0), stop=(c == KC - 1),
                        )
                    ot = obuf.tile([P, TQ], f16, tag="ot")
                    nc.vector.tensor_copy(out=ot[:], in_=ps3)
                    nc.sync.dma_start(
                        out=out[P * i : P * i + P, TQ * n : TQ * n + TQ],
                        in_=ot[:],
                    )

                p3_queue = []
                for J in range(NQ):
                    jsl = slice(TQ * J, TQ * J + TQ)
                    nt = 4 * J + 4
                    rbufJ = rbp.tile([1, HL * TQ], f32, tag="rb", name=f"rb{J}")
                    for h in range(HL):
                        o_ps = ops.tile([P, TQ], f32, tag="o")
                        r_ps = arps.tile([1, TQ], f32, tag="r")

                        def scores(t):
                            d = t - 4 * J
                            qs, ext = (QS[d], TQ - QS[d]) if d >= 0 else (0, TQ)
                            st = stps.tile([P, TQ], f32, tag="st")
                            nc.tensor.matmul(
                                st[:, qs : qs + ext],
                                lhsT=KT[:, h, P * t : P * t + P],
                                rhs=QT[:, h, TQ * J + qs : TQ * J + qs + ext],
                                start=True, stop=True,
                            )
                            f = fp.tile([P, TQ], f16, tag="f")
                            nc.scalar.activation(
                                out=f[:, :ext], in_=st[:, qs : qs + ext],
                                func=AF.Exp, scale=SCALE, bias=expb[:],
                            )
                            if d >= 0:
                                off = 384 - (P * d - qs)
                                nc.vector.tensor_mul(
                                    out=f[:, :ext], in0=f[:, :ext],
                                    in1=masks[:, off : off + ext],
                                )
                            e = ep.tile([P, TQ], bf16, tag="e")
                            nc.vector.tensor_mul(
                                out=e[:, :ext], in0=f[:, :ext], in1=f[:, :ext]
                            )
                            return (t, f, e, qs, ext)

                        def accum(unit):
                            t, f, e, qs, ext = unit
                            nc.tensor.matmul(
                                r_ps[:, qs : qs + ext],
                                lhsT=onesb[:],
                                rhs=e[:, :ext],
                                start=(t == 0), stop=(t == nt - 1),
                            )
                            nc.tensor.matmul(
                                o_ps[:, qs : qs + ext],
                                lhsT=V[:, t, P * h : P * h + P],
                                rhs=f[:, :ext],
                                start=(t == 0), stop=(t == nt - 1),
                            )

                        # software pipeline, depth 1: scores(t+1) then accum(t)
                        pend = scores(0)
                        for t in range(1, nt):
                            cur = scores(t)
                            accum(pend)
                            pend = cur
                        accum(pend)
                        # unnormalized o'/4 -> attnT; r row -> rbufJ
                        nc.vector.tensor_scalar(
                            out=attnT[:, h, jsl], in0=o_ps,
                            scalar1=0.25, scalar2=None, op0=OP.mult,
                        )
                        nc.vector.tensor_copy(
                            out=rbufJ[:, TQ * h : TQ * h + TQ], in_=r_ps
                        )
                        # fill PE slack with out-projection of finished quarters
                        if p3_queue:
                            p3_group(*p3_queue.pop(0))
                    # cr = rsqrt(r') for the 4 heads of this quarter
                    lnr = rbp.tile([1, HL * TQ], f32, tag="ln", name=f"ln{J}")
                    nc.scalar.activation(out=lnr[:], in_=rbufJ[:], func=AF.Ln)
                    crJ = crp.tile([1, HL * TQ], f16, tag="cr", name=f"cr{J}")
                    nc.scalar.activation(
                        out=crJ[:], in_=lnr[:], func=AF.Exp, scale=-0.5
                    )
                    # broadcast each head's cr row and normalize attnT
                    for h in range(HL):
                        rd = rowd.tile([1, TQ], f16, tag="rd")
                        nc.sync.dma_start(
                            out=rd, in_=crJ[:, TQ * h : TQ * h + TQ]
                        )
                        bap = _b.AP(
                            tensor=rd.tensor, offset=rd.offset,
                            ap=[[0, P], [1, TQ]],
                        )
                        cbt = cbtp.tile([P, TQ], f16, tag="cb")
                        nc.sync.dma_start(out=cbt[:], in_=bap)
                        nc.vector.tensor_mul(
                            out=attnT[:, h, jsl], in0=attnT[:, h, jsl],
                            in1=cbt[:],
                        )
                    for i in range(4 * J, 4 * J + 4):
                        for n in range(2):
                            p3_queue.append((i, n))
                # drain remaining out-projection groups (quarter 3)
                while p3_queue:
                    p3_group(*p3_queue.pop(0))

    _split_multi_waits(nc, mybir)
    return nc


def _core_inputs(inputs, core):
    """Per-core input map: host-side sharding, fp16 casts, gamma folding."""
    x = np.asarray(inputs["x"], dtype=np.float32)
    gamma = np.asarray(inputs["gamma"], dtype=np.float32)
    beta = np.asarray(inputs["beta"], dtype=np.float32)
    w_qkv = np.asarray(inputs["w_qkv"], dtype=np.float32)
    w_out = np.asarray(inputs["w_out"], dtype=np.float32)

    b, j = core // 2, core % 2
    sl = slice(512 * j, 512 * j + 512)

    def wslice(base):
        wsub = w_qkv[base : base + D][sl]  # [512 out, 1024 in]
        wg = wsub * gamma[None, :]
        bias = wsub @ beta  # [512]
        return np.ascontiguousarray(wg.T).astype(np.float16), bias.astype(np.float32)

    wq16, bq = wslice(0)
    wk16, bk = wslice(D)
    wv16, bvr = wslice(2 * D)
    wl16, bl = wslice(3 * D)
    wp16, bp = wslice(4 * D)

    bqk = np.stack(
        [bq[128 * t : 128 * t + 128] for t in range(4)]
        + [bk[128 * t : 128 * t + 128] for t in range(4)],
        axis=1,
    )
    blp = np.stack(
        [bl[128 * t : 128 * t + 128] for t in range(4)]
        + [bp[128 * t : 128 * t + 128] for t in range(4)],
        axis=1,
    )

    cols = np.r_[512 * j : 512 * j + 512, D + 512 * j : D + 512 * j + 512]
    wo_sel = w_out[:, cols].copy()
    wo_sel[:, 512:] *= 4.0  # attnT stores o'/4; fold the 4x back here
    wo16 = np.ascontiguousarray(wo_sel.T).astype(np.float16)

    kk = np.arange(P)[:, None]
    cc = np.arange(896)[None, :]
    mask16 = (cc >= kk + 384).astype(np.float16)

    return {
        "xT16": np.ascontiguousarray(x[b].T).astype(np.float16),
        "wq16": wq16,
        "wk16": wk16,
        "wv16": wv16,
        "wl16": wl16,
        "wp16": wp16,
        "wo16": wo16,
        "bqk": np.ascontiguousarray(bqk),
        "blp": np.ascontiguousarray(blp),
        "bv16": bvr.astype(np.float16)[None, :],
        "mask16": mask16,
    }


def _run(inputs, trace=False, trace_kwargs=None):
    from concourse.bass_utils import run_bass_kernel_spmd

    beta_zero = not np.any(np.asarray(inputs["beta"]))
    key = ("nc", beta_zero)
    if key not in _CACHED:
        _CACHED[key] = _build_nc(beta_zero)
    nc = _CACHED[key]
    in_maps = [_core_inputs(inputs, c) for c in range(8)]
    res = run_bass_kernel_spmd(
        nc, in_maps, core_ids=list(range(8)), trace=trace,
        **(trace_kwargs or {}),
    )
    x = np.asarray(inputs["x"], dtype=np.float32)
    out = np.empty((B, L, D), dtype=np.float32)
    for b in range(B):
        out[b] = x[b] + (
            res.results[2 * b]["out16"].astype(np.float32)
            + res.results[2 * b + 1]["out16"].astype(np.float32)
        )
    return out, res


def kernel(**inputs) -> np.ndarray:
    out, _ = _run(inputs, trace=False)
    return out
